# revision 8
# baseline (speedup 1.0000x reference)
"""BiAffineParser kernel, ARCH C: separable-polynomial gelu (no ACT tables).

gelu(u+v) over the L x L span grid is replaced by the rank-(D+1) expansion
    gelu(u+v) ~= sum_{d=0}^{D} a_d(u) * v^d
with the a_d fitted (jointly, Gaussian-weighted over the actual u/v input
distribution) as degree-P polynomials.  The whole (B,L,L,H) tensor never
exists: everything becomes 13*(D) standard 768-contraction matmuls

    logits[i,j,n] = sum_d sum_h (W2[h,n]*a_d(u_ih)) * v_jh^d    (+ b2)

Per core (8 cores = 4 batches x 2 i-halves, i=128 rows):
  PE : fp16 projections u,v; then for (d,c,n): lhsT = A_ndc [128h,128i]
       bf16 (FWL), rhs = v^d chunk [128h, 256j] -> psum[n] [128i, 256j].
       d=0 contracts against a ones-column into PS0 [128i, 13n].
  DVE: v-powers (bf16 2x), Horner for a_d on [128,768] tiles, and the
       390 fused (acc+c0)*W2n scale instructions (bf16 4x).
  ACT: psum->sbuf copies: v bf16, u+b1 (Identity, per-partition bias), and
       the final gather psum->[i,(j n)] with bias = PS0+b2 column.
  DMA: output written as 128 x 13.3KB contiguous runs (one DMA).
"""

import sys

if "/opt/trn_rl_repo" not in sys.path:
    sys.path.insert(0, "/opt/trn_rl_repo")

import numpy as np

B = 4
L = 256
H = 768
NH = 6           # 128-partition chunks of H
NL = 13          # num labels
IH = 128         # i rows per core
D = 4            # v-power degree (rank D+1)
P = 6            # a_d polynomial degree in u

# Joint Gaussian-weighted LSQ fit of gelu(u+v) ~ sum_d a_d(u) v^d on
# [-3.5,3.5]^2, weight std 0.46; a_d(u) = sum_e COEF[d][e] u^e.
COEF = [
    [9.97129631e-04, 5.00000000e-01, 3.94053208e-01, 3.79482311e-16, -5.88879354e-02, 3.91001485e-17, 5.10528002e-03],
    [5.00000000e-01, 7.67533617e-01, 8.53898299e-16, -2.19759016e-01, -4.00058135e-16, 2.77082020e-02, 2.13547260e-16],
    [3.85437677e-01, -3.71776249e-17, -3.53327579e-01, 9.10912825e-17, 1.06587861e-01, 3.93196254e-17, -1.17572027e-02],
    [-2.32260036e-16, -1.70735108e-01, 1.66850675e-16, 9.23606692e-02, -4.26542380e-16, -1.43037288e-02, 3.50284423e-16],
    [-4.31992818e-02, 9.41385229e-17, 7.65791672e-02, -1.24210831e-16, -2.93929971e-02, 2.83370653e-17, 3.62739056e-03],
]

_CACHE = {}


def _build(repeat=1):
    import concourse.mybir as mybir
    from concourse import bacc
    from concourse.tile import TileContext

    f32 = mybir.dt.float32
    f16 = mybir.dt.float16
    bf16 = mybir.dt.bfloat16
    IDENT = mybir.ActivationFunctionType.Identity
    COPY = mybir.ActivationFunctionType.Copy
    MULT = mybir.AluOpType.mult
    ADD = mybir.AluOpType.add

    nc = bacc.Bacc("TRN2", target_bir_lowering=False)

    xt_d = nc.dram_tensor("xt", [128, NH * L], f16, kind="ExternalInput")
    xts_d = nc.dram_tensor("xts", [128, NH * IH], f16, kind="ExternalInput")
    w1s_d = nc.dram_tensor("w1s", [128, NH * NH * 128], f16, kind="ExternalInput")
    w1e_d = nc.dram_tensor("w1e", [128, NH * NH * 128], f16, kind="ExternalInput")
    b1t_d = nc.dram_tensor("b1t", [128, NH], f32, kind="ExternalInput")
    w2c_d = nc.dram_tensor("w2c", [128, NH * NL], f32, kind="ExternalInput")
    w2b_d = nc.dram_tensor("w2b", [128, NH * NL], bf16, kind="ExternalInput")
    b2t_d = nc.dram_tensor("b2t", [1, NL], bf16, kind="ExternalInput")
    out_d = nc.dram_tensor("out", [IH, L, NL], f32, kind="ExternalOutput")

    with TileContext(nc) as tc:
        def body():
            with (
                tc.tile_pool(name="consts", bufs=1) as cp,
                tc.tile_pool(name="qp", bufs=1, space="PSUM") as qp,
                tc.tile_pool(name="ap", bufs=10) as apool,
                tc.tile_pool(name="w1p", bufs=1) as wp,
            ):
                # ---- loads (u-path first: XTS + W1S feed the Horner) -------
                XTSf = cp.tile([128, NH * IH], f16, tag="xtsf", name="XTSf")
                nc.sync.dma_start(out=XTSf, in_=xts_d[:, :])

                # W1 thirds as separate tiles (tile-granular deps), one DMA
                # each, so chunk-0 projections start after a third of a load.
                wh = NH * NH * 128 // 3
                w1t = {}

                def _w1third(dram, side, hf):
                    key = (side, hf)
                    if key not in w1t:
                        t = wp.tile([128, wh], f16, tag=f"w1{side}{hf}",
                                    name=f"W1{side}{hf}")
                        nc.sync.dma_start(
                            out=t, in_=dram[:, hf * wh:(hf + 1) * wh])
                        w1t[key] = t
                    return w1t[key]

                def load_ws(k):
                    t = _w1third(w1s_d, "s", k // 2)
                    off = (k % 2) * NH * 128
                    return t[:, off:off + NH * 128]

                def load_we(k):
                    t = _w1third(w1e_d, "e", k // 2)
                    off = (k % 2) * NH * 128
                    return t[:, off:off + NH * 128]

                load_ws(0)
                B1T = cp.tile([128, NH], f32, tag="b1t", name="B1T")
                nc.sync.dma_start(out=B1T, in_=b1t_d[:, :])
                load_ws(2)
                load_ws(4)
                XTf = cp.tile([128, NH * L], f16, tag="xtf", name="XTf")
                nc.sync.dma_start(out=XTf, in_=xt_d[:, :])
                XT = [XTf[:, h * L:(h + 1) * L] for h in range(NH)]
                W2C = cp.tile([128, NH * NL], f32, tag="w2c", name="W2C")
                nc.sync.dma_start(out=W2C, in_=w2c_d[:, :])
                W2B = cp.tile([128, NH * NL], bf16, tag="w2b", name="W2B")
                nc.sync.dma_start(out=W2B, in_=w2b_d[:, :])
                B2R = cp.tile([1, NL], bf16, tag="b2r", name="B2R")
                nc.sync.dma_start(out=B2R, in_=b2t_d[:, :])
                XTS = [XTSf[:, h * IH:(h + 1) * IH] for h in range(NH)]
                ONESI = cp.tile([1, 128], bf16, tag="onesi", name="ONESI")
                nc.vector.memset(ONESI, 1.0)

                ONE1 = cp.tile([128, 1], bf16, tag="one1", name="ONE1")
                nc.vector.memset(ONE1, 1.0)

                # psum: 7 banks of [128,512] f32; PSA[p] holds logits for
                # n=2p ([:, :256]) and n=2p+1 ([:, 256:]).  PS0 (d=0 per-i
                # sums) gets its own bank so its readers don't serialize on
                # the n=12 accumulation (tile-granular dependency tracking).
                PSA = [qp.tile([128, 2 * L], f32, tag=f"psa{p}", name=f"PSA{p}")
                       for p in range(7)]
                PS0 = qp.tile([128, NL], f32, tag="ps0", name="PS0")

                def ps_n(n):
                    p, hf = n // 2, n % 2
                    return PSA[p][:, hf * L:hf * L + L]

                # ---- projections (PE, fp16): u first, then v ---------------
                VB = cp.tile([128, NH * L], bf16, tag="vb", name="VB")
                UB = cp.tile([128, NH * 128], bf16, tag="ub", name="UB")
                for k in range(NH):
                    W1Sk = load_ws(k)
                    if k == 0:
                        load_we(0)
                    pxs = PSA[0 + 2 * (k % 2)][:, :IH]
                    for h in range(NH):
                        nc.tensor.matmul(
                            pxs, lhsT=W1Sk[:, h * 128:(h + 1) * 128], rhs=XTS[h],
                            start=(h == 0), stop=(h == NH - 1),
                            skip_group_check=True,
                        )
                    nc.scalar.activation(
                        out=UB[:, k * IH:(k + 1) * IH], in_=pxs, func=IDENT,
                        bias=B1T[:, k:k + 1],
                    )
                for k in range(NH):
                    W1Ek = load_we(k)
                    pxe = PSA[1 + 2 * (k % 2)][:, :L]
                    for h in range(NH):
                        nc.tensor.matmul(
                            pxe, lhsT=W1Ek[:, h * 128:(h + 1) * 128], rhs=XT[h],
                            start=(h == 0), stop=(h == NH - 1),
                            skip_group_check=True,
                        )
                    nc.scalar.activation(
                        out=VB[:, k * L:(k + 1) * L], in_=pxe, func=COPY)

                # ---- a_d evaluation (DVE), parity-reduced ------------------
                # Each fitted a_d is even or odd in u (+const):
                #   d even: a_d = c0 [+ c1 u] + s(c2 + s(c4 + s c6)),  s = u^2
                #   d odd : a_d = c0 + u(c1 + s(c3 + s c5))
                # a_0 -> its own tile AC0 (feeds the early d=0 matmuls);
                # a_1..a_D -> ACCcat[:, (d-1)*768:d*768], W2-scaled later.
                AC0 = cp.tile([128, NH * 128], bf16, tag="ac0", name="AC0")
                ACCcat = cp.tile([128, D * NH * 128], bf16, tag="accat",
                                 name="ACCcat")
                S2 = cp.tile([128, NH * 128], bf16, tag="s2", name="S2")
                nc.vector.tensor_tensor(out=S2, in0=UB, in1=UB, op=MULT)
                for d in (1, 2, 3, 4, 0):
                    c = COEF[d]
                    dst = AC0 if d == 0 else ACCcat[:, (d - 1) * 768:d * 768]
                    t = cp.tile([128, NH * 128], bf16, tag=f"tmp{d}",
                                name=f"TMP{d}")
                    if d % 2 == 1:
                        nc.vector.tensor_scalar(
                            out=t, in0=S2, scalar1=float(c[5]),
                            scalar2=float(c[3]), op0=MULT, op1=ADD)
                        nc.vector.tensor_tensor(out=t, in0=t, in1=S2, op=MULT)
                        nc.vector.tensor_scalar(
                            out=t, in0=t, scalar1=float(c[1]),
                            scalar2=None, op0=ADD)
                        nc.vector.tensor_tensor(out=t, in0=t, in1=UB, op=MULT)
                        nc.vector.tensor_scalar(
                            out=dst, in0=t, scalar1=float(c[0]),
                            scalar2=None, op0=ADD)
                    else:
                        nc.vector.tensor_scalar(
                            out=t, in0=S2, scalar1=float(c[6]),
                            scalar2=float(c[4]), op0=MULT, op1=ADD)
                        nc.vector.tensor_tensor(out=t, in0=t, in1=S2, op=MULT)
                        nc.vector.tensor_scalar(
                            out=t, in0=t, scalar1=float(c[2]),
                            scalar2=None, op0=ADD)
                        if abs(c[1]) > 1e-9:
                            w = cp.tile([128, NH * 128], bf16, tag="w0",
                                        name=f"W0_{d}")
                            nc.vector.tensor_scalar(
                                out=w, in0=UB, scalar1=float(c[1]),
                                scalar2=float(c[0]), op0=MULT, op1=ADD)
                            nc.vector.tensor_tensor(out=t, in0=t, in1=S2, op=MULT)
                            nc.vector.tensor_tensor(out=dst, in0=t, in1=w, op=ADD)
                        else:
                            nc.vector.tensor_tensor(out=t, in0=t, in1=S2, op=MULT)
                            nc.vector.tensor_scalar(
                                out=dst, in0=t, scalar1=float(c[0]),
                                scalar2=None, op0=ADD)

                # ---- d=0 pass: PS0[i,n] = b2 + sum_h W2[h,n] a_0(u[h,i]) ---
                # rhs = W2 itself (bf16), so no per-n builds and PS0 finishes
                # right after the a_0 Horner -> gathers never tail-block.
                nc.tensor.matmul(
                    PS0, lhsT=ONESI, rhs=B2R,
                    start=True, stop=False, skip_group_check=True,
                )
                for c in range(NH):
                    nc.tensor.matmul(
                        PS0, lhsT=AC0[:, c * 128:(c + 1) * 128],
                        rhs=W2B[:, c * NL:(c + 1) * NL],
                        start=False, stop=(c == NH - 1),
                        skip_group_check=True,
                    )
                PS0S = cp.tile([128, NL], f32, tag="ps0s", name="PS0S")
                nc.scalar.activation(out=PS0S, in_=PS0, func=COPY)

                # ---- v powers (Pool, bf16) ---------------------------------
                VP = {1: VB}
                for d in range(2, D + 1):
                    t = cp.tile([128, NH * L], bf16, tag=f"vp{d}", name=f"VP{d}")
                    nc.gpsimd.tensor_tensor(
                        out=t, in0=VP[d - 1], in1=VB, op=MULT)
                    VP[d] = t

                ACv = ACCcat.rearrange("p (d q) -> p d q", d=D)

                # ---- A builds (DVE) + d-outer matmul passes ----------------
                # d=1 uses VP[1]=VB directly, so the whole first pass overlaps
                # the Pool-side power computation.
                # d=1 MMs are build-rate-limited on DVE, so the d=2 pass (no
                # builds needed) is software-pipelined into the build phase
                # with a 2-n-group lag to keep PE fed.
                AT = {}
                all_nc = [(n, c) for n in range(NL) for c in range(NH)]
                LAG = 2 * NH

                def mm_d2(n, c):
                    nc.tensor.matmul(
                        ps_n(n), lhsT=AT[n, c][:, 128:256],
                        rhs=VP[2][:, c * L:(c + 1) * L],
                        start=False, stop=False,
                        skip_group_check=True,
                    )

                for idx, (n, c) in enumerate(all_nc):
                    Acat = apool.tile([128, D * 128], bf16, tag="at",
                                      bufs=NL * NH, name=f"A_{c}_{n}")
                    # last builds go to the (by then idle) GPSIMD engine to
                    # shorten the DVE build stream gating the MM pipeline
                    beng = nc.gpsimd if n >= NL - 2 else nc.vector
                    beng.tensor_scalar(
                        out=Acat.rearrange("p (d q) -> p d q", d=D),
                        in0=ACv[:, :, c * 128:(c + 1) * 128],
                        scalar1=W2C[:, c * NL + n:c * NL + n + 1],
                        scalar2=None, op0=MULT)
                    AT[n, c] = Acat
                    nc.tensor.matmul(
                        ps_n(n), lhsT=Acat[:, 0:128], rhs=VB[:, c * L:(c + 1) * L],
                        start=(c == 0 and n % 2 == 0), stop=False,
                        skip_group_check=True,
                    )
                    if idx >= LAG:
                        mm_d2(*all_nc[idx - LAG])
                for idx in range(len(all_nc) - LAG, len(all_nc)):
                    mm_d2(*all_nc[idx])

                # ---- gather + bias (ACT), per-n after its d=D pass ---------
                # Two half-tiles so each output DMA waits only its own half.
                JH = L // 2
                Th = [cp.tile([128, JH * NL], f32, tag=f"T{j}", name=f"T{j}")
                      for j in range(2)]
                tvh = [t.rearrange("p (j n) -> p j n", n=NL) for t in Th]

                for d in range(3, D + 1):
                    for n in range(NL):
                        for c in range(NH):
                            nc.tensor.matmul(
                                ps_n(n),
                                lhsT=AT[n, c][:, (d - 1) * 128:d * 128],
                                rhs=VP[d][:, c * L:(c + 1) * L],
                                start=False,
                                stop=(c == NH - 1 and d == D),
                                skip_group_check=True,
                            )
                        if d == D:
                            for jh in range(2):
                                nc.scalar.activation(
                                    out=tvh[jh][:, :, n:n + 1],
                                    in_=ps_n(n)[:, jh * JH:(jh + 1) * JH]
                                    .rearrange("p (j o) -> p j o", o=1),
                                    func=IDENT,
                                    bias=PS0S[:, n:n + 1],
                                )
                for jh in range(2):
                    eng = nc.sync if jh == 0 else nc.scalar
                    eng.dma_start(
                        out=out_d[:, jh * JH:(jh + 1) * JH, :],
                        in_=tvh[jh],
                    )

        if repeat == 1:
            body()
        else:
            with tc.For_i(0, repeat, 1):
                body()

    nc.compile()
    return nc


def _get_program(repeat=1):
    if repeat not in _CACHE:
        _CACHE[repeat] = _build(repeat)
    return _CACHE[repeat]


def make_in_maps(hidden_states, W1, b1, W2, b2):
    hidden_states = np.asarray(hidden_states, dtype=np.float32)
    W1 = np.asarray(W1, dtype=np.float32)
    b1 = np.asarray(b1, dtype=np.float32)
    W2 = np.asarray(W2, dtype=np.float32)
    b2 = np.asarray(b2, dtype=np.float32)

    def w1_prep(w):
        # [(c p), (k kk)] -> [p, (k c kk)]: per-out-chunk slabs side by side
        return np.ascontiguousarray(
            w.reshape(NH, 128, NH, 128).transpose(1, 2, 0, 3)
            .reshape(128, NH * NH * 128)
        ).astype(np.float16)

    w1s = w1_prep(W1[:H])
    w1e = w1_prep(W1[H:])
    w2c = np.ascontiguousarray(
        W2.reshape(NH, 128, NL).transpose(1, 0, 2).reshape(128, NH * NL)
    )
    import ml_dtypes

    w2b = w2c.astype(ml_dtypes.bfloat16)
    b1t = np.ascontiguousarray(b1.reshape(NH, 128).T)
    b2t = np.ascontiguousarray(b2.reshape(1, NL)).astype(ml_dtypes.bfloat16)

    in_maps = []
    for core in range(8):
        b, ih = core // 2, core % 2
        xt = np.ascontiguousarray(
            hidden_states[b].reshape(L, NH, 128).transpose(2, 1, 0)
            .reshape(128, NH * L)
        ).astype(np.float16)
        xts = np.ascontiguousarray(
            hidden_states[b][ih * IH:(ih + 1) * IH]
            .reshape(IH, NH, 128).transpose(2, 1, 0).reshape(128, NH * IH)
        ).astype(np.float16)
        in_maps.append(
            {
                "xt": xt,
                "xts": xts,
                "w1s": w1s,
                "w1e": w1e,
                "b1t": b1t,
                "w2c": w2c,
                "w2b": w2b,
                "b2t": b2t,
            }
        )
    return in_maps


def kernel(hidden_states, W1, b1, W2, b2):
    from concourse.bass_utils import run_bass_kernel_spmd

    nc = _get_program()
    in_maps = make_in_maps(hidden_states, W1, b1, W2, b2)
    res = run_bass_kernel_spmd(nc, in_maps, core_ids=list(range(8)))

    out = np.empty((B, L, L, NL), dtype=np.float32)
    for core in range(8):
        b, ih = core // 2, core % 2
        out[b, ih * IH:(ih + 1) * IH] = res.results[core]["out"]
    return out
